# revision 8
# baseline (speedup 1.0000x reference)
# Contrastive-loss kernel for Trainium2 (Bass/Tile), 8-core data-parallel.
#
# Math (see reference):
#   S[i,j]     = (x_i . y_j) / T
#   denom[i,k] = sum_{j<=k} exp(S[i,j]) + (B-1-k)
#   loss       = sum_{i,k} log(denom[i,k]) - sum_i (B-i) * S[i,i]
#
# Device formulation per core (512 rows of x, full y):
#   - matmul (bf16) -> PSUM S_raw tiles [128, 512]
#   - ACT exp with scale=1/T : expS = exp(S_raw/T)            (PSUM -> SBUF)
#   - DVE tensor_tensor_scan: denom[k] = B + cumsum(expS - 1)
#         state = (expS[k] + state) + (-1), initial = B
#     (identical to cumE[k] + (B-1-k))
#   - ACT ln with accum_out: per-partition sum_k log(denom)
#   - diag: partial[p] = lnsum[p] + sum_d(xpre ⊙ y_row)   with
#         xpre = -(B-i)/T * x  (host-precomputed)  == lnsum - (B-i)*S_ii
#   - host sums the 8 x [128, 4] partials -> scalar loss.

import numpy as np
import ml_dtypes

B = 4096
D = 256
NCORES = 8
ROWS = B // NCORES      # 512 rows per core
P = 128                 # SBUF partitions
RT = ROWS // P          # 4 row-tiles per core
JT = 512                # matmul moving free-dim tile
HALF = 2048             # psum/exp chunk (4 banks)
TEMP = 0.07

_CACHE = {}
LAST_RESULTS = None     # BassKernelResults of the most recent run (for test.py)


def _build():
    from contextlib import ExitStack

    import concourse.bacc as bacc
    import concourse.mybir as mybir
    import concourse.tile as tile

    dt = mybir.dt
    Act = mybir.ActivationFunctionType
    Alu = mybir.AluOpType

    nc = bacc.Bacc(
        "TRN2", target_bir_lowering=False, debug=False, num_devices=NCORES
    )

    xT = nc.dram_tensor("xT", (D, ROWS), dt.bfloat16, kind="ExternalInput").ap()
    yT = nc.dram_tensor("yT", (D, B), dt.bfloat16, kind="ExternalInput").ap()
    xpre = nc.dram_tensor("xpre", (ROWS, D), dt.float32, kind="ExternalInput").ap()
    ysh = nc.dram_tensor("ysh", (ROWS, D), dt.float32, kind="ExternalInput").ap()
    out = nc.dram_tensor("partial", (P, RT), dt.float32, kind="ExternalOutput").ap()

    with tile.TileContext(nc) as tc, ExitStack() as ctx:
        wpool = ctx.enter_context(tc.tile_pool(name="weights", bufs=1))
        psum = ctx.enter_context(tc.tile_pool(name="psum", bufs=2, space="PSUM"))
        big = ctx.enter_context(tc.tile_pool(name="big", bufs=3))
        small = ctx.enter_context(tc.tile_pool(name="small", bufs=4))

        # x^T shard: two K-chunks of [128, 512] bf16
        xT_t = []
        for kc in range(2):
            xt = wpool.tile([P, ROWS], dt.bfloat16, name=f"xTs{kc}")
            nc.sync.dma_start(out=xt, in_=xT[kc * P:(kc + 1) * P, :])
            xT_t.append(xt)
        # y^T: two K-chunks of [128, 4096] bf16; split loads so DMA queues
        # parallelize and matmuls can start on the first column chunk.
        yT_t = []
        for kc in range(2):
            yt = wpool.tile([P, B], dt.bfloat16, name=f"yTs{kc}")
            yT_t.append(yt)
        for j in range(4):
            for kc in range(2):
                nc.sync.dma_start(
                    out=yT_t[kc][:, j * 1024:(j + 1) * 1024],
                    in_=yT[kc * P:(kc + 1) * P, j * 1024:(j + 1) * 1024],
                )

        negones = wpool.tile([P, B], dt.float32)
        nc.gpsimd.memset(negones, -1.0)

        resall = wpool.tile([P, RT], dt.float32)

        # Phase A: matmul -> exp -> scan (+ diag term) per row-tile.
        # All Exp ACTIVATEs are emitted before any Ln so the static ACT
        # stream switches table sets exactly once (saves ~5us of
        # ACT_TABLE_LOAD ping-pong).
        denoms = []
        dterms = []
        for m in range(RT):
            expS = big.tile([P, B], dt.float32, tag="expS")
            for h in range(2):
                ps = psum.tile([P, HALF], dt.float32, tag="ps")
                for jb in range(HALF // JT):
                    j0 = h * HALF + jb * JT
                    for kc in range(2):
                        nc.tensor.matmul(
                            ps[:, jb * JT:(jb + 1) * JT],
                            xT_t[kc][:, m * P:(m + 1) * P],
                            yT_t[kc][:, j0:j0 + JT],
                            start=(kc == 0),
                            stop=(kc == 1),
                        )
                nc.scalar.activation(
                    out=expS[:, h * HALF:(h + 1) * HALF],
                    in_=ps,
                    func=Act.Exp,
                    scale=1.0 / TEMP,
                )

            denom = big.tile([P, B], dt.float32, tag=f"denom{m}", bufs=1)
            nc.vector.tensor_tensor_scan(
                out=denom,
                data0=expS,
                data1=negones,
                initial=float(B),
                op0=Alu.add,
                op1=Alu.add,
            )
            denoms.append(denom)

            xp = small.tile([P, D], dt.float32, tag="xp")
            nc.sync.dma_start(out=xp, in_=xpre[m * P:(m + 1) * P, :])
            yp = small.tile([P, D], dt.float32, tag="yp")
            nc.sync.dma_start(out=yp, in_=ysh[m * P:(m + 1) * P, :])
            prod = small.tile([P, D], dt.float32, tag="prod")
            dterm = small.tile([P, 1], dt.float32, tag=f"dterm{m}", bufs=1)
            # dterm = sum_d(xpre * y) = -(B-i)*S_ii  (xpre negated on host)
            nc.vector.scalar_tensor_tensor(
                out=prod,
                in0=xp,
                scalar=1.0,
                in1=yp,
                op0=Alu.mult,
                op1=Alu.mult,
                accum_out=dterm,
            )
            dterms.append(dterm)

        # Phase B: ln (one table load) + combine.
        for m in range(RT):
            lnout = big.tile([P, B], dt.float32, tag="expS")
            lsum = small.tile([P, 1], dt.float32, tag="lsum")
            nc.scalar.activation(
                out=lnout, in_=denoms[m], func=Act.Ln, accum_out=lsum
            )
            # resall[:, m] = lsum + dterm = lnsum - (B-i)*S_ii
            nc.vector.tensor_add(resall[:, m:m + 1], lsum, dterms[m])

        nc.sync.dma_start(out=out, in_=resall)

    nc.compile()
    return nc


def _get_nc():
    if "nc" not in _CACHE:
        _CACHE["nc"] = _build()
    return _CACHE["nc"]


def kernel(x: np.ndarray, y: np.ndarray) -> np.ndarray:
    global LAST_RESULTS
    from concourse import bass_utils

    nc = _get_nc()

    x = np.asarray(x, dtype=np.float32)
    y = np.asarray(y, dtype=np.float32)

    yT_full = np.ascontiguousarray(y.T.astype(ml_dtypes.bfloat16))  # [D, B]
    nhits = (B - np.arange(B, dtype=np.float64)) / TEMP             # (B-i)/T
    in_maps = []
    for c in range(NCORES):
        sl = slice(c * ROWS, (c + 1) * ROWS)
        xs = x[sl]                                                   # [ROWS, D]
        in_maps.append(
            {
                "xT": np.ascontiguousarray(xs.T.astype(ml_dtypes.bfloat16)),
                "yT": yT_full,
                "xpre": np.ascontiguousarray(
                    (-nhits[sl, None] * xs.astype(np.float64)).astype(np.float32)
                ),
                "ysh": np.ascontiguousarray(y[sl]),
            }
        )

    res = bass_utils.run_bass_kernel_spmd(
        nc, in_maps, core_ids=list(range(NCORES))
    )
    LAST_RESULTS = res

    total = 0.0
    for c in range(NCORES):
        total += res.results[c]["partial"].astype(np.float64).sum()
    return np.asarray(total, dtype=np.float32)


# revision 9
# speedup vs baseline: 1.1324x; 1.1324x over previous
# Contrastive-loss kernel for Trainium2 (Bass/Tile), 8-core data-parallel.
#
# Math (see reference):
#   S[i,j]     = (x_i . y_j) / T
#   denom[i,k] = sum_{j<=k} exp(S[i,j]) + (B-1-k)
#   loss       = sum_{i,k} log(denom[i,k]) - sum_i (B-i) * S[i,i]
#
# Device formulation per core (512 rows of x, full y):
#   - matmul (bf16) -> PSUM S_raw tiles [128, 512]
#   - ACT exp with scale=1/T : expS = exp(S_raw/T)            (PSUM -> SBUF)
#   - DVE tensor_tensor_scan: denom[k] = B + cumsum(expS - 1)
#         state = (expS[k] + state) + (-1), initial = B
#     (identical to cumE[k] + (B-1-k))
#   - ACT ln with accum_out: per-partition sum_k log(denom)
#   - diag: partial[p] = lnsum[p] + sum_d(xpre ⊙ y_row)   with
#         xpre = -(B-i)/T * x  (host-precomputed)  == lnsum - (B-i)*S_ii
#   - host sums the 8 x [128, 4] partials -> scalar loss.

import numpy as np
import ml_dtypes

B = 4096
D = 256
NCORES = 8
ROWS = B // NCORES      # 512 rows per core
P = 128                 # SBUF partitions
RT = ROWS // P          # 4 row-tiles per core
JT = 512                # matmul moving free-dim tile
HALF = 2048             # psum/exp chunk (4 banks)
TEMP = 0.07

_CACHE = {}
LAST_RESULTS = None     # BassKernelResults of the most recent run (for test.py)


def _build():
    from contextlib import ExitStack

    import concourse.bacc as bacc
    import concourse.mybir as mybir
    import concourse.tile as tile

    dt = mybir.dt
    Act = mybir.ActivationFunctionType
    Alu = mybir.AluOpType

    nc = bacc.Bacc(
        "TRN2", target_bir_lowering=False, debug=False, num_devices=NCORES
    )

    xT = nc.dram_tensor("xT", (D, ROWS), dt.bfloat16, kind="ExternalInput").ap()
    yT = nc.dram_tensor("yT", (D, B), dt.bfloat16, kind="ExternalInput").ap()
    xpre = nc.dram_tensor("xpre", (ROWS, D), dt.float32, kind="ExternalInput").ap()
    ysh = nc.dram_tensor("ysh", (ROWS, D), dt.float32, kind="ExternalInput").ap()
    out = nc.dram_tensor("partial", (P, RT), dt.float32, kind="ExternalOutput").ap()

    with tile.TileContext(nc) as tc, ExitStack() as ctx:
        wpool = ctx.enter_context(tc.tile_pool(name="weights", bufs=1))
        psum = ctx.enter_context(tc.tile_pool(name="psum", bufs=2, space="PSUM"))
        big = ctx.enter_context(tc.tile_pool(name="big", bufs=3))
        small = ctx.enter_context(tc.tile_pool(name="small", bufs=4))

        from concourse.tile import add_dep_helper

        # x^T shard: two K-chunks of [128, 512] bf16 (first: tiny, gates all
        # matmuls).
        xT_t = []
        for kc in range(2):
            xt = wpool.tile([P, ROWS], dt.bfloat16, name=f"xTs{kc}")
            nc.sync.dma_start(out=xt, in_=xT[kc * P:(kc + 1) * P, :])
            xT_t.append(xt)
        # y^T: two K-chunks of [128, 4096] bf16. 4 big DMAs (SP descriptor
        # issue costs ~700ns each, so few+large beats many+small), ordered
        # low columns first so the first matmuls can start early.
        yT_t = []
        for kc in range(2):
            yt = wpool.tile([P, B], dt.bfloat16, name=f"yTs{kc}")
            yT_t.append(yt)
        for j in range(2):
            for kc in range(2):
                nc.sync.dma_start(
                    out=yT_t[kc][:, j * HALF:(j + 1) * HALF],
                    in_=yT[kc * P:(kc + 1) * P, j * HALF:(j + 1) * HALF],
                )

        negones = wpool.tile([P, HALF], dt.float32)
        nc.gpsimd.memset(negones, -1.0)

        resall = wpool.tile([P, RT], dt.float32)

        # Phase A: matmul -> exp -> scan (+ diag term) per row-tile half.
        # All Exp ACTIVATEs are emitted before any Ln so the static ACT
        # stream switches table sets exactly once.
        denoms = []
        dterms = []
        exp_insts = []
        for m in range(RT):
            denom = big.tile([P, B], dt.float32, tag="denom", bufs=3)
            for h in range(2):
                ps = psum.tile([P, HALF], dt.float32, tag="ps")
                for jb in range(HALF // JT):
                    j0 = h * HALF + jb * JT
                    for kc in range(2):
                        nc.tensor.matmul(
                            ps[:, jb * JT:(jb + 1) * JT],
                            xT_t[kc][:, m * P:(m + 1) * P],
                            yT_t[kc][:, j0:j0 + JT],
                            start=(kc == 0),
                            stop=(kc == 1),
                        )
                expS = big.tile([P, HALF], dt.float32, tag="expS", bufs=4)
                ei = nc.scalar.activation(
                    out=expS,
                    in_=ps,
                    func=Act.Exp,
                    scale=1.0 / TEMP,
                )
                exp_insts.append(ei)
                # denom[:, h] = B + cumsum(expS - 1), carried across halves
                nc.vector.tensor_tensor_scan(
                    out=denom[:, h * HALF:(h + 1) * HALF],
                    data0=expS,
                    data1=negones,
                    initial=(
                        float(B) if h == 0 else denom[:, HALF - 1:HALF]
                    ),
                    op0=Alu.add,
                    op1=Alu.add,
                )
            denoms.append(denom)

            xp = small.tile([P, D], dt.float32, tag="xp")
            nc.gpsimd.dma_start(out=xp, in_=xpre[m * P:(m + 1) * P, :])
            yp = small.tile([P, D], dt.float32, tag="yp")
            nc.gpsimd.dma_start(out=yp, in_=ysh[m * P:(m + 1) * P, :])
            prod = small.tile([P, D], dt.float32, tag="prod")
            dterm = small.tile([P, 1], dt.float32, tag=f"dterm{m}", bufs=1)
            # dterm = sum_d(xpre * y) = -(B-i)*S_ii  (xpre negated on host)
            nc.vector.scalar_tensor_tensor(
                out=prod,
                in0=xp,
                scalar=1.0,
                in1=yp,
                op0=Alu.mult,
                op1=Alu.mult,
                accum_out=dterm,
            )
            dterms.append(dterm)

        # Phase B: ln in-place over denom (one table load) + combine.
        for m in range(RT):
            lsum = small.tile([P, 1], dt.float32, tag="lsum")
            li = nc.scalar.activation(
                out=denoms[m], in_=denoms[m], func=Act.Ln, accum_out=lsum
            )
            # Pin ACT order: every Ln after the last Exp, so the table set
            # switches exactly once.
            try:
                add_dep_helper(li.ins, exp_insts[-1].ins, False, "act set order")
            except Exception:
                pass
            # resall[:, m] = lsum + dterm = lnsum - (B-i)*S_ii
            nc.vector.tensor_add(resall[:, m:m + 1], lsum, dterms[m])

        nc.gpsimd.dma_start(out=out, in_=resall)

    nc.compile()
    return nc


def _get_nc():
    if "nc" not in _CACHE:
        _CACHE["nc"] = _build()
    return _CACHE["nc"]


def kernel(x: np.ndarray, y: np.ndarray) -> np.ndarray:
    global LAST_RESULTS
    from concourse import bass_utils

    nc = _get_nc()

    x = np.asarray(x, dtype=np.float32)
    y = np.asarray(y, dtype=np.float32)

    yT_full = np.ascontiguousarray(y.T.astype(ml_dtypes.bfloat16))  # [D, B]
    nhits = (B - np.arange(B, dtype=np.float64)) / TEMP             # (B-i)/T
    in_maps = []
    for c in range(NCORES):
        sl = slice(c * ROWS, (c + 1) * ROWS)
        xs = x[sl]                                                   # [ROWS, D]
        in_maps.append(
            {
                "xT": np.ascontiguousarray(xs.T.astype(ml_dtypes.bfloat16)),
                "yT": yT_full,
                "xpre": np.ascontiguousarray(
                    (-nhits[sl, None] * xs.astype(np.float64)).astype(np.float32)
                ),
                "ysh": np.ascontiguousarray(y[sl]),
            }
        )

    res = bass_utils.run_bass_kernel_spmd(
        nc, in_maps, core_ids=list(range(NCORES))
    )
    LAST_RESULTS = res

    total = 0.0
    for c in range(NCORES):
        total += res.results[c]["partial"].astype(np.float64).sum()
    return np.asarray(total, dtype=np.float32)
